# revision 68
# baseline (speedup 1.0000x reference)
"""Trainium2 Bass kernel for nn_MultiHeadAttention_85229331022244.

Computation (per batch b):
  xh = x.reshape(B,T,64,16); q/k/v = per-head 64x64 projections of xh
  q,k: interleaved RoPE over the FULL 1024-dim feature axis
  scores = q @ k.T / sqrt(1024)  (single attention map over full D)
  causal softmax; y = attn @ v

Sharding: core i -> batch i//2, q-tile parity i%2 (even/odd 128-row q-tiles
interleaved between the two cores of a batch).  Every core runs an identical
program; causality differences are carried in per-core mask data.

Device layout trick: heads are reordered even-first and paired so the
projections become 8 block-diagonal 128x128 matmuls that produce K^T/Q^T
directly in [feature-on-partition, token] layout, with RoPE partner features
living in chunk c and c+4 at the same partition index.

Q^T/K^T are stored fp8e4m3 in per-DR-pair tiles [128, 2, 1024] so the score
matmuls run in DoubleRow perf mode (two 128-feature chunks contracted per
instruction) and each matmul only depends on the two chunks it reads.  V and
the attention probabilities stay bf16 (fp8 V fails the accuracy budget).
"""

import math
from contextlib import ExitStack

import numpy as np
import ml_dtypes

import concourse.bass as bass
import concourse.mybir as mybir
import concourse.tile as tile
from concourse import bacc
from concourse.bass import ts, ds
from concourse.masks import make_identity

BF16 = ml_dtypes.bfloat16
F8 = ml_dtypes.float8_e4m3

D_MODEL = 1024
N_HEADS = 16
HEAD_D = 64
ROPE_BASE = 10000.0
GAMMA = 1.0 / math.sqrt(D_MODEL)

# head pairs per 128-row chunk; chunks 0-3 = even heads, 4-7 = odd heads
HEAD_PAIRS = [(0, 2), (4, 6), (8, 10), (12, 14), (1, 3), (5, 7), (9, 11), (13, 15)]


def _feature_perm():
    """perm[c*128 + p] = original feature index for kernel row (c, p)."""
    perm = np.zeros(1024, dtype=np.int64)
    for c, (ha, hb) in enumerate(HEAD_PAIRS):
        for p in range(128):
            h = ha if p < 64 else hb
            perm[c * 128 + p] = (p % 64) * 16 + h
    return perm


PERM = _feature_perm()
INV_PERM = np.argsort(PERM)


def _block_weights(w):
    """w: (64, 64, 16) -> (128, 8*128) block-diag per chunk, bf16."""
    out = np.zeros((8, 128, 128), dtype=np.float32)
    for c, (ha, hb) in enumerate(HEAD_PAIRS):
        out[c, :64, :64] = w[:, :, ha]
        out[c, 64:, 64:] = w[:, :, hb]
    # -> [128, 8, 128] -> [128, 1024]
    return np.ascontiguousarray(out.transpose(1, 0, 2).reshape(128, 1024)).astype(BF16)


def _rope_tables(T):
    """cos/sin tables [4, 128, T] bf16 for chunks 0-3 (and partners 4-7)."""
    p = np.arange(128)
    cos = np.zeros((4, 128, T), dtype=np.float32)
    sin = np.zeros((4, 128, T), dtype=np.float32)
    t = np.arange(T, dtype=np.float64)
    for c in range(4):
        f = (p % 64) * 8 + (2 * c + p // 64)  # [128]
        inv_freq = ROPE_BASE ** (-f.astype(np.float64) / 512.0)  # [128]
        ang = inv_freq[:, None] * t[None, :]  # [128, T]
        cos[c] = np.cos(ang).astype(np.float32)
        sin[c] = np.sin(ang).astype(np.float32)
    return cos.astype(BF16), sin.astype(BF16)


def _n_stripes(j):
    return (2 * j + 2 + 3) // 4


def _last_width(j):
    nblk = 2 * j + 2
    w = nblk - 4 * (_n_stripes(j) - 1)
    return w * 128  # 256 (j even) or 512 (j odd)


def _masks_for_core(parity):
    """[128, 2*512] fp32 additive masks: cols 0-511 = even-j, 512-1023 = odd-j.

    For q-tile j (global tile G = 2j+parity), the last key stripe starts at
    k0 = 4*(n_stripes(j)-1)*128.  tq - tk = (G*128 + p) - (k0 + c):
      j even: k0 = 2j*128    -> unmasked iff c <= parity*128 + p
      j odd:  k0 = (2j-2)*128 -> unmasked iff c <= (parity+2)*128 + p
    Both are independent of j.
    """
    m = np.zeros((128, 2, 512), dtype=np.float32)
    p = np.arange(128)[:, None]
    c = np.arange(512)[None, :]
    m[:, 0, :] = np.where(c <= parity * 128 + p, 0.0, -1e9)
    m[:, 1, :] = np.where(c <= (parity + 2) * 128 + p, 0.0, -1e9)
    return m.reshape(128, 1024)


def build_nc(T, NQ):
    """Build the (identical-on-all-cores) Bass program.

    T:  total key length (keys 0..T-1 resident per core)
    NQ: number of 128-row query tiles handled by this core
    Requires: max blocks = 2*(NQ-1)+2 <= T//128, T % 512 == 0, NQ % 4 == 0.
    """
    assert T % 1024 == 0 and NQ % 8 == 0
    assert 2 * NQ <= T // 128
    n_kv_stripes2 = T // 1024
    dt = mybir.dt

    nc = bacc.Bacc("TRN2", target_bir_lowering=False)
    # x and rope tables with the even/odd partner chunk (resp. cos/sin)
    # stacked on a middle axis so each (pair, stripe) needs ONE dma.
    xpk = nc.dram_tensor("xpk", [4, 128, 2, T], dt.bfloat16, kind="ExternalInput")
    xqk = nc.dram_tensor("xqk", [4, 128, 2, NQ * 128], dt.bfloat16,
                         kind="ExternalInput")
    w2q = nc.dram_tensor("w2q", [128, 1024], dt.bfloat16, kind="ExternalInput")
    w2k = nc.dram_tensor("w2k", [128, 1024], dt.bfloat16, kind="ExternalInput")
    w2v = nc.dram_tensor("w2v", [128, 1024], dt.bfloat16, kind="ExternalInput")
    csk = nc.dram_tensor("csk", [4, 128, 2, T], dt.bfloat16, kind="ExternalInput")
    csq = nc.dram_tensor("csq", [4, 128, 2, NQ * 128], dt.bfloat16,
                         kind="ExternalInput")
    masks = nc.dram_tensor("masks", [128, 1024], dt.float32, kind="ExternalInput")
    y = nc.dram_tensor("y", [NQ * 128, 1024], dt.bfloat16, kind="ExternalOutput")

    with tile.TileContext(nc) as tc, ExitStack() as ctx:
        const = ctx.enter_context(tc.tile_pool(name="const", bufs=1))
        kv = ctx.enter_context(tc.tile_pool(name="kv", bufs=1))
        qpool = ctx.enter_context(tc.tile_pool(name="qpool", bufs=2))
        xpool = ctx.enter_context(tc.tile_pool(name="xpool", bufs=6))
        cspool = ctx.enter_context(tc.tile_pool(name="cspool", bufs=4))
        rtmp = ctx.enter_context(tc.tile_pool(name="rtmp", bufs=4))
        ppool = ctx.enter_context(tc.tile_pool(name="ppool", bufs=2))
        ptpool = ctx.enter_context(tc.tile_pool(name="ptpool", bufs=2))
        ypool = ctx.enter_context(tc.tile_pool(name="ypool", bufs=2))
        lpool = ctx.enter_context(tc.tile_pool(name="lpool", bufs=2))
        # PSUM bank budget (8): S (2) + pt_ps (1) + pe + po + VV + YL + YH
        psum = ctx.enter_context(tc.tile_pool(name="psum", bufs=2, space="PSUM"))
        psumB = ctx.enter_context(tc.tile_pool(name="psumB", bufs=1, space="PSUM"))
        psum1 = ctx.enter_context(tc.tile_pool(name="psum1", bufs=1, space="PSUM"))

        # constants
        ident = const.tile([128, 128], dt.bfloat16, tag="ident", name="ident")
        make_identity(nc, ident)
        wq_sb = const.tile([128, 1024], dt.bfloat16, tag="wq", name="wq")
        wk_sb = const.tile([128, 1024], dt.bfloat16, tag="wk", name="wk")
        wv_sb = const.tile([128, 1024], dt.bfloat16, tag="wv", name="wv")
        nc.sync.dma_start(wq_sb[:], w2q[:])
        nc.sync.dma_start(wk_sb[:], w2k[:])
        nc.sync.dma_start(wv_sb[:], w2v[:])
        mask_sb = const.tile([128, 1024], dt.float32, tag="mask", name="mask")

        # resident K^T (fp8, per-DR-pair tiles so score matmuls only wait on
        # the two chunks they read) and V
        KT = [
            [kv.tile([128, 2, 1024], dt.float8e4, tag=f"kt{s}_{i}",
                     name=f"kt{s}_{i}") for i in range(4)]
            for s in range(n_kv_stripes2)
        ]
        V = [
            kv.tile([128, 8, 1024], dt.bfloat16, tag=f"v{s}", name=f"v{s}")
            for s in range(n_kv_stripes2)
        ]
        # Q^T streamed per 1024-token stripe (fp8, per-DR-pair tiles)
        QT = {}

        def proj_rope_pair(cp, h, sl, x_dram, w_sb, cos_dram, sin_dram, out8,
                           do_v, v_stripe):
            """One head-pair (chunks cp, cp+4) over one 512-token half-stripe.

            sl: slice in the source token axis; out8: [128, 8, 1024] fp8 tile,
            written at free offset h*512.
            """
            hs = ds(h * 512, 512)
            xab = xpool.tile([128, 2, 512], dt.bfloat16, tag="xab", name="xab")
            nc.sync.dma_start(xab[:], x_dram[cp][:, :, sl])
            cs = cspool.tile([128, 2, 512], dt.bfloat16, tag=f"cs{cp}",
                             name=f"cs{cp}")
            nc.sync.dma_start(cs[:], cos_dram[cp][:, :, sl])
            xa, xb = xab[:, 0, :], xab[:, 1, :]
            cos, sin = cs[:, 0, :], cs[:, 1, :]

            pe = psum1.tile([128, 512], dt.float32, tag="pe", name="pe")
            po = psum1.tile([128, 512], dt.float32, tag="po", name="po")
            nc.tensor.matmul(pe[:], lhsT=w_sb[:, ds(cp * 128, 128)], rhs=xa,
                             start=True, stop=True)
            nc.tensor.matmul(po[:], lhsT=w_sb[:, ds((cp + 4) * 128, 128)],
                             rhs=xb, start=True, stop=True)
            ke = rtmp.tile([128, 512], dt.bfloat16, tag="ke", name="ke")
            ko = rtmp.tile([128, 512], dt.bfloat16, tag="ko", name="ko")
            nc.scalar.copy(ke[:], pe[:])
            nc.scalar.copy(ko[:], po[:])
            ta = rtmp.tile([128, 512], dt.bfloat16, tag="ta", name="ta")
            tb = rtmp.tile([128, 512], dt.bfloat16, tag="tb", name="tb")
            re = rtmp.tile([128, 512], dt.bfloat16, tag="re", name="re")
            ro = rtmp.tile([128, 512], dt.bfloat16, tag="ro", name="ro")
            # re = ke*cos - ko*sin ; ro = ke*sin + ko*cos  (DVE bf16 2x mode;
            # the bf16->fp8 cast rides a SWDGE SBUF->SBUF DMA, not an engine)
            nc.vector.tensor_mul(ta[:], ke[:], cos)
            nc.vector.tensor_mul(tb[:], ko[:], sin)
            nc.vector.tensor_sub(re[:], ta[:], tb[:])
            ta2 = rtmp.tile([128, 512], dt.bfloat16, tag="ta", name="ta")
            tb2 = rtmp.tile([128, 512], dt.bfloat16, tag="tb", name="tb")
            nc.vector.tensor_mul(ta2[:], ke[:], sin)
            nc.vector.tensor_mul(tb2[:], ko[:], cos)
            nc.vector.tensor_add(ro[:], ta2[:], tb2[:])
            nc.gpsimd.dma_start(out8[cp // 2][:, cp % 2, hs], re[:])
            nc.gpsimd.dma_start(out8[(cp + 4) // 2][:, cp % 2, hs], ro[:])

            if do_v:
                va = psum1.tile([128, 4, 128], dt.float32, tag="VV", name="va")
                for sub in range(4):
                    nc.tensor.matmul(
                        va[:, sub, :], lhsT=xab[:, 0, ts(sub, 128)],
                        rhs=wv_sb[:, ds(cp * 128, 128)],
                        start=True, stop=True,
                    )
                nc.any.tensor_copy(
                    v_stripe[:, ds(h * 4, 4), ds(cp * 128, 128)], va[:])
                vb = psum1.tile([128, 4, 128], dt.float32, tag="VV", name="vb")
                for sub in range(4):
                    nc.tensor.matmul(
                        vb[:, sub, :], lhsT=xab[:, 1, ts(sub, 128)],
                        rhs=wv_sb[:, ds((cp + 4) * 128, 128)],
                        start=True, stop=True,
                    )
                nc.any.tensor_copy(
                    v_stripe[:, ds(h * 4, 4), ds((cp + 4) * 128, 128)], vb[:])

        def emit_kv_stripe(s2, h, pairs=(0, 1, 2, 3)):
            sl = ds(s2 * 1024 + h * 512, 512)
            for cp in pairs:
                proj_rope_pair(cp, h, sl, xpk, wk_sb, csk, None, KT[s2],
                               True, V[s2])

        def emit_q_half(qs2, h, pairs=(0, 1, 2, 3)):
            if qs2 not in QT:
                QT[qs2] = [
                    qpool.tile([128, 2, 1024], dt.float8e4, tag=f"qt{i}",
                               name=f"qt{i}") for i in range(4)
                ]
            sl = ds(qs2 * 1024 + h * 512, 512)
            for cp in pairs:
                proj_rope_pair(cp, h, sl, xqk, wq_sb, csq, None, QT[qs2],
                               False, None)

        # ---- Phases B+C interleaved: Q^T stripe then its 4 q-tiles ----
        def emit_q_tile(j):
            nst = _n_stripes(j)
            nblk = 2 * j + 2
            y_lo = psum1.tile([128, 512], dt.float32, tag="YL", name="YL")
            y_hi = psum1.tile([128, 512], dt.float32, tag="YH", name="YH")
            l_parts = lpool.tile([128, 16], dt.float32, tag="lp", name="lp")
            qs, qoff = j // 8, (j % 8) * 128
            for s in range(nst):
                w = 512 if s < nst - 1 else _last_width(j)
                S = psum.tile([128, 512], dt.float32, tag="A", name="A")
                # order (0,2,1,3): DR tiles 0 and 2 are completed by rope
                # pairs 0-1 alone, so half the accumulation can start early
                for n, i in enumerate((0, 2, 1, 3)):
                    nc.tensor.matmul(
                        S[:, :w],
                        lhsT=QT[qs][i][:, :, ds(qoff, 128)],
                        rhs=KT[s // 2][i][:, :, ds((s % 2) * 512, w)],
                        start=(n == 0), stop=(n == 3),
                        perf_mode=mybir.MatmulPerfMode.DoubleRow,
                    )
                if s == nst - 1:
                    nc.vector.tensor_add(S[:, :w], S[:, :w],
                                         mask_sb[:, ds((j % 2) * 512, w)])
                P = ppool.tile([128, 512], dt.bfloat16, tag="p", name="p")
                nc.scalar.activation(
                    P[:, :w], S[:, :w], mybir.ActivationFunctionType.Exp,
                    scale=GAMMA, accum_out=l_parts[:, ds(s, 1)],
                )
                nb = w // 128
                pt_ps = psumB.tile([128, 512], dt.bfloat16, tag="B", name="ptps")
                for b in range(nb):
                    nc.tensor.transpose(pt_ps[:, ts(b, 128)], P[:, ts(b, 128)],
                                        ident[:])
                pt = ptpool.tile([128, 512], dt.bfloat16, tag="pt", name="pt")
                nc.vector.tensor_copy(pt[:, :w], pt_ps[:, :w])
                for b in range(nb):
                    blk = s * 4 + b
                    vs = V[blk // 8]
                    nc.tensor.matmul(y_lo[:], lhsT=pt[:, ts(b, 128)],
                                     rhs=vs[:, blk % 8, 0:512],
                                     start=(blk == 0), stop=(blk == nblk - 1))
                    nc.tensor.matmul(y_hi[:], lhsT=pt[:, ts(b, 128)],
                                     rhs=vs[:, blk % 8, 512:1024],
                                     start=(blk == 0), stop=(blk == nblk - 1))
            lsum = lpool.tile([128, 1], dt.float32, tag="ls", name="ls")
            linv = lpool.tile([128, 1], dt.float32, tag="li", name="li")
            nc.vector.tensor_reduce(lsum[:], l_parts[:, :nst],
                                    mybir.AxisListType.X, mybir.AluOpType.add)
            nc.vector.reciprocal(linv[:], lsum[:])
            y_sb = ypool.tile([128, 1024], dt.bfloat16, tag="y", name="y")
            nc.scalar.activation(y_sb[:, 0:512], y_lo[:],
                                 mybir.ActivationFunctionType.Copy,
                                 scale=linv[:])
            nc.scalar.activation(y_sb[:, 512:1024], y_hi[:],
                                 mybir.ActivationFunctionType.Copy,
                                 scale=linv[:])
            nc.sync.dma_start(y[ts(j, 128), :], y_sb[:])

        # Interleave: KV stripe s2+1 is emitted in the MIDDLE of group s2's
        # q-tiles so its dma/proj/rope runs under their PE work; Q-stripe
        # production two tiles before its first consumer; the mask load is
        # emitted late so it doesn't delay the first x loads.
        # Interleave the first q-half and kv-half pair-by-pair: tile 0's
        # first score matmuls need q AND k rope pairs 0-1, so alternating
        # sides makes them ready after 4 pair-halves instead of 6.
        for cp in range(4):
            emit_q_half(0, 0, pairs=(cp,))
            emit_kv_stripe(0, 0, pairs=(cp,))
        nc.sync.dma_start(mask_sb[:], masks[:])
        emit_kv_stripe(0, 1)
        # Q half-stripe (qs2, h) is produced one tile-group before its first
        # consumer (tile 8*qs2 + 4*h) so it stays off the startup critical path.
        qhalf_sched = {}
        for qs2 in range((NQ + 7) // 8):
            for h in range(2):
                first_tile = 8 * qs2 + 4 * h
                if first_tile > 0:
                    qhalf_sched[max(0, first_tile - 4 + 2)] = (qs2, h)
        for s2 in range(n_kv_stripes2):
            for idx, j in enumerate(range(4 * s2, 4 * s2 + 4)):
                if j >= NQ:
                    continue
                if j in qhalf_sched:
                    emit_q_half(*qhalf_sched[j])
                if idx in (0, 1) and s2 + 1 < n_kv_stripes2:
                    emit_kv_stripe(s2 + 1, idx)
                emit_q_tile(j)

    nc.compile()
    return nc


# ------------------------- host side -------------------------


def _pair_stack(a):
    """[8, 128, n] -> [4, 128, 2, n] pairing chunk cp with cp+4."""
    return np.ascontiguousarray(
        np.stack([a[0:4], a[4:8]], axis=2))


def prep_core_inputs(xb, w2q, w2k, w2v, cs_t, parity, NQ, T):
    """Inputs for one core: batch slice xb (T, 1024) fp32, parity 0/1.

    cs_t: [4, 128, 2, T] combined cos/sin table (bf16).
    """
    q_tiles = [2 * j + parity for j in range(NQ)]
    xpT = xb.T[PERM].reshape(8, 128, T).astype(BF16)
    cols = np.concatenate([np.arange(G * 128, (G + 1) * 128) for G in q_tiles])
    return {
        "xpk": _pair_stack(xpT),
        "xqk": _pair_stack(xpT[:, :, cols]),
        "w2q": w2q,
        "w2k": w2k,
        "w2v": w2v,
        "csk": cs_t,
        "csq": np.ascontiguousarray(cs_t[:, :, :, cols]),
        "masks": _masks_for_core(parity),
    }


def _pair_unstack(a):
    """[4, 128, 2, n] -> [8, 128, n]."""
    return np.concatenate([a[:, :, 0, :], a[:, :, 1, :]], axis=0)


def core_model(inp, NQ):
    """Numpy model of what one core's program computes (fp32 math, for tests)."""
    T = inp["xpk"].shape[3]
    xpT = _pair_unstack(inp["xpk"]).astype(np.float32)
    xqT = _pair_unstack(inp["xqk"]).astype(np.float32)
    cosk = inp["csk"][:, :, 0, :].astype(np.float32)
    sink = inp["csk"][:, :, 1, :].astype(np.float32)
    cosq = inp["csq"][:, :, 0, :].astype(np.float32)
    sinq = inp["csq"][:, :, 1, :].astype(np.float32)
    w2q = inp["w2q"].astype(np.float32).reshape(128, 8, 128)
    w2k = inp["w2k"].astype(np.float32).reshape(128, 8, 128)
    w2v = inp["w2v"].astype(np.float32).reshape(128, 8, 128)

    def proj_T(xT, w2):  # -> [8, 128, n]
        return np.stack([w2[:, c, :].T @ xT[c] for c in range(8)])

    def rope(zT, cos, sin, to_f8):
        out = np.empty_like(zT)
        for c in range(4):
            e, o = zT[c], zT[c + 4]
            out[c] = e * cos[c] - o * sin[c]
            out[c + 4] = e * sin[c] + o * cos[c]
        if to_f8:
            out = out.astype(BF16).astype(F8).astype(np.float32)
        else:
            out = out.astype(BF16).astype(np.float32)
        return out

    kT = rope(proj_T(xpT, w2k), cosk, sink, True).reshape(1024, T)
    qT = rope(proj_T(xqT, w2q), cosq, sinq, True).reshape(1024, NQ * 128)
    v = np.concatenate([w2v[:, c, :].T @ xpT[c] for c in range(8)], axis=0).T
    v = v.astype(BF16).astype(np.float32)  # [T, 1024]

    masks = inp["masks"].reshape(128, 2, 512)
    y = np.zeros((NQ * 128, 1024), dtype=np.float32)
    for j in range(NQ):
        nblk = 2 * j + 2
        q = qT[:, j * 128:(j + 1) * 128].T  # [128, 1024]
        keys = kT[:, : nblk * 128]
        S = q @ keys  # [128, nblk*128]
        k0 = 4 * (_n_stripes(j) - 1) * 128
        S[:, k0:] += masks[:, j % 2, : nblk * 128 - k0]
        P = np.exp(GAMMA * S)
        yj = (P.astype(BF16).astype(np.float32) @ v[: nblk * 128])
        yj = yj / P.sum(1, keepdims=True)
        y[j * 128:(j + 1) * 128] = yj.astype(BF16).astype(np.float32)
    return y


_NC_CACHE = {}
last_in_maps = None


def kernel(x, w_q, w_k, w_v):
    global last_in_maps
    from concourse.bass_utils import run_bass_kernel_spmd

    B, T, D = x.shape
    assert (B, T, D) == (4, 4096, 1024)
    NQ = 16
    x = np.asarray(x, dtype=np.float32)
    w2q = _block_weights(np.asarray(w_q, dtype=np.float32))
    w2k = _block_weights(np.asarray(w_k, dtype=np.float32))
    w2v = _block_weights(np.asarray(w_v, dtype=np.float32))
    cos_t, sin_t = _rope_tables(T)
    cs_t = np.ascontiguousarray(
        np.stack([cos_t[:4], sin_t[:4]], axis=2))  # [4,128,2,T]

    in_maps = []
    for core in range(8):
        b, parity = core // 2, core % 2
        in_maps.append(
            prep_core_inputs(x[b], w2q, w2k, w2v, cs_t, parity, NQ, T)
        )
    last_in_maps = in_maps

    key = (T, NQ)
    if key not in _NC_CACHE:
        _NC_CACHE[key] = build_nc(T, NQ)
    nc = _NC_CACHE[key]

    res = run_bass_kernel_spmd(nc, in_maps, core_ids=list(range(8)))
    out = np.zeros((B, T, D), dtype=np.float32)
    for core in range(8):
        b, parity = core // 2, core % 2
        yk = res.results[core]["y"].astype(np.float32).reshape(NQ, 128, D)
        for j in range(NQ):
            G = 2 * j + parity
            out[b, G * 128:(G + 1) * 128, :] = yk[j][:, INV_PERM]
    return out
